# revision 15
# baseline (speedup 1.0000x reference)
"""Trainium2 Bass kernel for ECCConv + GATConv GNN (nn_Net_5918464934506).

kernel(**inputs) takes full unsharded inputs, distributes across 8 NeuronCores,
and returns the full [N, 1] output.

Sharding: edges partitioned by dst-node ownership (node slabs of N/8 per core);
host gathers endpoint features per edge (per sharding hint). Segment-sum done
on-device via fp32 DMA scatter-add into each core's local node slab. GAT phase
uses fixed-degree padded slots with free-dim softmax reductions.
"""
import os
import sys

for _p in (
    "/root/.axon_site",
    "/root/.axon_site/_ro/trn_rl_repo",
    "/root/.axon_site/_ro/pypackages",
    "/opt/trn_rl_repo",
):
    if os.path.isdir(_p) and _p not in sys.path:
        sys.path.append(_p)

import numpy as np

import concourse.bass as bass
import concourse.mybir as mybir
import concourse.tile as tile
import concourse.bacc as bacc
from concourse.bass_utils import run_bass_kernel_spmd

F32 = mybir.dt.float32
F32R = mybir.dt.float32r
BF16 = mybir.dt.bfloat16
I16 = mybir.dt.int16
NPBF16 = mybir.dt.np(BF16)

N_CORES = 8
N = 50000
F_IN = 16
F_OUT = 64
S = 4
E_TOTAL = 200000
KN = 32

NPC = N // N_CORES          # nodes per core
NT = (NPC + 127) // 128     # node tiles per core
NPC_PAD = NT * 128

CHUNK = 512                 # edges per compute chunk
GRP = 2048                  # edges per DMA/z group (4 chunks)
SCATTER_CHUNKS = 10         # chunks per scatter-add batch


def _round_up(x, m):
    return (x + m - 1) // m * m


def _dram_ap(handle, offset, pairs):
    t = handle.tensor if hasattr(handle, "tensor") else handle
    return bass.AP(t, offset, [list(p) for p in pairs])


def build_phase1(EPC, use_b1, use_b2, use_ecc_bias):
    """ECC conv + xp projection. Per-core inputs; same program all cores.

    Edges are bucketed per 128-node dst tile with uniform capacity C =
    EPC // NT (multiple of 128). Segment-sum is one-hot matmuls fused with
    the root-kernel matmul in PSUM per node tile.
    """
    nc = bacc.Bacc(None, target_bir_lowering=False, debug=False)

    KR = 17 if use_ecc_bias else 16  # augmented root matmul K
    C = EPC // NT
    MPT = C // 128  # msg chunks (128 edges) per node tile

    e_t = nc.dram_tensor("e_t", [S, EPC], F32R, kind="ExternalInput")
    xs_t = nc.dram_tensor("xs_t", [F_IN, EPC], BF16, kind="ExternalInput")
    dloc = nc.dram_tensor("dloc", [128, EPC // 128], F32, kind="ExternalInput")
    iota_in = nc.dram_tensor("iota_in", [128, 128], BF16, kind="ExternalInput")
    xown_t = nc.dram_tensor("xown_t", [KR, NPC_PAD], F32, kind="ExternalInput")
    w1rep = nc.dram_tensor("w1rep", [S, 128], F32R, kind="ExternalInput")
    w2r = nc.dram_tensor("w2r", [128, 4 * F_OUT], BF16, kind="ExternalInput")
    b1rep = nc.dram_tensor("b1rep", [128, 1], F32, kind="ExternalInput")
    b2r = nc.dram_tensor("b2r", [F_IN, F_OUT], BF16, kind="ExternalInput")
    rootk = nc.dram_tensor("rootk", [KR, F_OUT], F32, kind="ExternalInput")
    gkrep = nc.dram_tensor("gkrep", [128, F_OUT], F32, kind="ExternalInput")

    xp_out = nc.dram_tensor("xp_out", [128, NT], F32, kind="ExternalOutput")
    x1p_out = nc.dram_tensor(
        "x1p_out", [128, NT, F_OUT], F32, kind="ExternalOutput")

    # edge groups of GRP, plus a tail so any EPC multiple of 128 works
    groups = []
    pos = 0
    while pos < EPC:
        groups.append((pos, min(GRP, EPC - pos)))
        pos += groups[-1][1]

    with tile.TileContext(nc) as tc:
        with (
            tc.tile_pool(name="consts", bufs=1) as consts,
            tc.tile_pool(name="stream", bufs=3) as stream,
            tc.tile_pool(name="xrep", bufs=8) as xrep_pool,
            tc.tile_pool(name="work", bufs=3) as work,
            tc.tile_pool(name="zpool", bufs=8) as zpool,
            tc.tile_pool(name="msg", bufs=1) as msgpool,
            tc.tile_pool(name="oh", bufs=8) as ohpool,
            tc.tile_pool(name="node", bufs=4) as nodepool,
            tc.tile_pool(name="ps", bufs=6, space="PSUM") as ps,
            tc.tile_pool(name="psagg", bufs=2, space="PSUM") as psagg,
        ):
            # ---- constants to SBUF ----
            w1_sb = consts.tile([S, 128], F32R, tag="w1")
            nc.sync.dma_start(w1_sb[:], w1rep[:, :])
            w2_sb = consts.tile([128, 4 * F_OUT], BF16, tag="w2")
            nc.sync.dma_start(w2_sb[:], w2r[:, :])
            b1_sb = consts.tile([128, 1], F32, tag="b1")
            nc.sync.dma_start(b1_sb[:], b1rep[:, :])
            b2_sb = consts.tile([F_IN, F_OUT], BF16, tag="b2")
            nc.sync.dma_start(b2_sb[:], b2r[:, :])
            rootk_sb = consts.tile([KR, F_OUT], F32, tag="rootk")
            nc.sync.dma_start(rootk_sb[:], rootk[:, :])
            gk_sb = consts.tile([128, F_OUT], F32, tag="gk")
            nc.sync.dma_start(gk_sb[:], gkrep[:, :])
            xown_sb = consts.tile([KR, NPC_PAD], F32, tag="xown")
            nc.sync.dma_start(xown_sb[:], xown_t[:, :])
            dloc_sb = consts.tile([128, EPC // 128], F32, tag="dloc")
            nc.sync.dma_start(dloc_sb[:], dloc[:, :])
            iota_sb = consts.tile([128, 128], BF16, tag="iota")
            nc.sync.dma_start(iota_sb[:], iota_in[:, :])

            msg_sb = msgpool.tile([128, EPC // 128, F_OUT], BF16, tag="msg")
            xp_sb = consts.tile([128, NT], F32, tag="xp")
            x1p_sb = msgpool.tile([128, NT, F_OUT], F32, tag="x1p")

            # ---- edge phase: msg for every (padded) edge ----
            for e0, glen in groups:
                e_sb = stream.tile([S, GRP], F32R, tag="e")
                nc.sync.dma_start(e_sb[:, :glen], e_t[:, e0:e0 + glen])
                if use_b2:
                    xs_sb = stream.tile([F_IN, GRP], BF16, tag="xs")
                    nc.sync.dma_start(xs_sb[:, :glen], xs_t[:, e0:e0 + glen])

                # x replicated across (f', k) partition pairs: partition
                # p = f'*KN + k reads x row (4c + f').
                xrep = []
                for c in range(4):
                    xr = xrep_pool.tile([128, GRP], BF16, tag="xrep")
                    src_ap = _dram_ap(
                        xs_t, (4 * c) * EPC + e0,
                        [[EPC, 4], [0, KN], [1, glen]],
                    )
                    nc.sync.dma_start(xr[:, :glen], src_ap)
                    xrep.append(xr)

                # h = relu(e @ W1 + b1), replicated 4x along partitions
                h_sb = work.tile([128, GRP], BF16, tag="h")
                q0 = 0
                while q0 < glen:
                    qlen = min(CHUNK, glen - q0)
                    hp = ps.tile([128, CHUNK], F32, tag="ps")
                    nc.tensor.matmul(
                        hp[:, :qlen], w1_sb[:], e_sb[:, q0:q0 + qlen],
                    )
                    if use_b1:
                        nc.scalar.activation(
                            h_sb[:, q0:q0 + qlen], hp[:, :qlen],
                            mybir.ActivationFunctionType.Relu, bias=b1_sb[:],
                        )
                    else:
                        nc.scalar.activation(
                            h_sb[:, q0:q0 + qlen], hp[:, :qlen],
                            mybir.ActivationFunctionType.Relu,
                        )
                    q0 += qlen

                # z chunks: z_c[(f',k), e] = x[4c+f', e] * h[k, e]
                zt = []
                for c in range(4):
                    z = zpool.tile([128, GRP], BF16, tag="z")
                    nc.vector.tensor_mul(
                        z[:, :glen], xrep[c][:, :glen], h_sb[:, :glen])
                    zt.append(z)

                # msg[e, :] = z_e @ W2r (+ x_e @ B2r): edge-major PSUM out
                for gg in range(glen // 128):
                    mp = ps.tile([128, F_OUT], F32, tag="ps")
                    for c in range(4):
                        nc.tensor.matmul(
                            mp[:],
                            zt[c][:, gg * 128:(gg + 1) * 128],
                            w2_sb[:, c * F_OUT:(c + 1) * F_OUT],
                            start=(c == 0),
                            stop=(c == 3 and not use_b2),
                        )
                    if use_b2:
                        nc.tensor.matmul(
                            mp[:],
                            xs_sb[:, gg * 128:(gg + 1) * 128],
                            b2_sb[:],
                            start=False, stop=True,
                        )
                    nc.scalar.copy(msg_sb[:, e0 // 128 + gg, :], mp[:])

            # ---- per node tile: agg (one-hot matmuls) + root mm in PSUM,
            # then x1 = relu(.), xp = x1 @ gk
            for t in range(NT):
                ap_ps = psagg.tile([128, F_OUT], F32, tag="agg")
                for m in range(MPT):
                    ch = t * MPT + m
                    oh = ohpool.tile([128, 128], BF16, tag="oh")
                    nc.gpsimd.tensor_scalar(
                        oh[:], iota_sb[:], dloc_sb[:, ch:ch + 1], None,
                        mybir.AluOpType.is_equal,
                    )
                    nc.tensor.matmul(
                        ap_ps[:], oh[:], msg_sb[:, ch, :],
                        start=(m == 0), stop=False,
                    )
                nc.tensor.matmul(
                    ap_ps[:], xown_sb[:, t * 128:(t + 1) * 128], rootk_sb[:],
                    start=False, stop=True,
                )
                nc.scalar.copy(x1p_sb[:, t, :], ap_ps[:])
                t1 = nodepool.tile([128, F_OUT], F32, tag="t1")
                nc.vector.scalar_tensor_tensor(
                    t1[:], ap_ps[:], 0.0, gk_sb[:],
                    mybir.AluOpType.max, mybir.AluOpType.mult,
                )
                nc.vector.reduce_sum(
                    xp_sb[:, t:t + 1], t1[:], axis=mybir.AxisListType.X,
                )

            nc.sync.dma_start(xp_out[:, :], xp_sb[:])
            nc.sync.dma_start(x1p_out[:, :, :], x1p_sb[:])

    nc.compile()
    return nc


def build_phase2(D, cs, cn, gat_bias):
    """GAT attention with fixed-degree padded slots."""
    nc = bacc.Bacc(None, target_bir_lowering=False, debug=False)
    W = NT * D

    xps = nc.dram_tensor("xps", [128, W], F32, kind="ExternalInput")
    mask = nc.dram_tensor("mask", [128, W], F32, kind="ExternalInput")
    xpdcs = nc.dram_tensor("xpdcs", [128, NT], F32, kind="ExternalInput")
    sig_out = nc.dram_tensor("sig_out", [128, NT], F32, kind="ExternalOutput")

    def bcast(ap2d):
        return bass.AP(ap2d.tensor, ap2d.offset, list(ap2d.ap) + [[0, D]])

    with tile.TileContext(nc) as tc:
        with tc.tile_pool(name="p", bufs=1) as pool:
            xps_sb = pool.tile([128, W], F32, tag="xps")
            nc.sync.dma_start(xps_sb[:], xps[:, :])
            mask_sb = pool.tile([128, W], F32, tag="mask")
            nc.sync.dma_start(mask_sb[:], mask[:, :])
            xpd_sb = pool.tile([128, NT], F32, tag="xpd")
            nc.sync.dma_start(xpd_sb[:], xpdcs[:, :])

            def v3(t):  # [128, W] tile -> [128, NT, D] view
                return t[:].rearrange("p (t d) -> p t d", d=D)

            spre = pool.tile([128, W], F32, tag="spre")
            nc.vector.scalar_tensor_tensor(
                v3(spre), xps_sb[:], float(cn), bcast(xpd_sb[:]),
                mybir.AluOpType.mult, mybir.AluOpType.add,
            )
            score = pool.tile([128, W], F32, tag="score")
            # lrelu(x) = max(0.2*x, x)
            nc.vector.scalar_tensor_tensor(
                score[:], spre[:], 0.2, spre[:],
                mybir.AluOpType.mult, mybir.AluOpType.max,
            )
            smax = pool.tile([128, NT], F32, tag="smax")
            nc.vector.reduce_max(
                smax[:], v3(score), axis=mybir.AxisListType.X,
            )
            sub = pool.tile([128, W], F32, tag="sub")
            nc.vector.tensor_sub(v3(sub), v3(score), bcast(smax[:]))
            ex = pool.tile([128, W], F32, tag="ex")
            nc.scalar.activation(
                ex[:], sub[:], mybir.ActivationFunctionType.Exp,
            )
            exm = pool.tile([128, W], F32, tag="exm")
            nc.vector.tensor_mul(exm[:], ex[:], mask_sb[:])
            denom = pool.tile([128, NT], F32, tag="denom")
            nc.vector.reduce_sum(
                denom[:], v3(exm), axis=mybir.AxisListType.X,
            )
            nm = pool.tile([128, W], F32, tag="nm")
            nc.vector.tensor_mul(nm[:], exm[:], xps_sb[:])
            numer = pool.tile([128, NT], F32, tag="numer")
            nc.vector.reduce_sum(
                numer[:], v3(nm), axis=mybir.AxisListType.X,
            )
            rec = pool.tile([128, NT], F32, tag="rec")
            nc.vector.reciprocal(rec[:], denom[:])
            av = pool.tile([128, NT], F32, tag="av")
            nc.vector.tensor_mul(av[:], numer[:], rec[:])
            sig = pool.tile([128, NT], F32, tag="sig")
            nc.scalar.activation(
                sig[:], av[:], mybir.ActivationFunctionType.Sigmoid,
                bias=float(gat_bias),
            )
            nc.sync.dma_start(sig_out[:, :], sig[:])

    nc.compile()
    return nc


def _prep_phase1_inputs(x, e, edge_src, edge_dst, W1, b1, W2, b2,
                        root_kernel, ecc_bias, use_ecc_bias):
    """Host-side sharding: bucket edges by dst node-tile, gather x[src]."""
    gtile = edge_dst // 128  # global 128-node tile id (aligned w/ core slabs
    # only when NPC % 128 == 0; cores own tiles [c*NT, (c+1)*NT) of padded ids)
    core = edge_dst // NPC
    loc_tile = (edge_dst - core * NPC) // 128
    bucket = core * NT + loc_tile
    bcounts = np.bincount(bucket, minlength=N_CORES * NT)
    C = max(_round_up(int(bcounts.max()), 128), 128)
    EPC = NT * C

    # rank of each edge within its bucket
    order = np.argsort(bucket, kind="stable")
    bstarts = np.zeros(N_CORES * NT + 1, np.int64)
    np.cumsum(bcounts, out=bstarts[1:])
    rank = np.empty(len(order), np.int64)
    rank[order] = np.arange(len(order)) - bstarts[bucket[order]]
    pos = loc_tile * C + rank  # position within the core's edge array

    KR = 17 if use_ecc_bias else 16

    # shared constants
    pidx = np.arange(128)
    w1rep = np.ascontiguousarray(W1[:, pidx % KN])              # [4, 128]
    b1rep = np.ascontiguousarray(b1[pidx % KN])[:, None]        # [128, 1]
    # w2r[p=(f',k), c*64+o] = W2[k, (4c+f')*64+o]
    w2r = np.empty((128, 4 * F_OUT), np.float32)
    for c in range(4):
        f = 4 * c + pidx // KN
        w2r[:, c * F_OUT:(c + 1) * F_OUT] = W2[pidx % KN, :].reshape(
            128, F_IN, F_OUT)[pidx, f, :]
    w2r = w2r.astype(NPBF16)
    b2r = b2.reshape(F_IN, F_OUT).astype(NPBF16)
    rootk = root_kernel.astype(np.float32)
    if use_ecc_bias:
        rootk = np.concatenate([rootk, ecc_bias[None, :]], axis=0)
    gkrep = None  # filled by caller with gat kernel

    iota = np.broadcast_to(
        np.arange(128, dtype=np.float32)[None, :], (128, 128))
    iota = np.ascontiguousarray(iota).astype(NPBF16)

    in_maps = []
    for c in range(N_CORES):
        sel = np.where(core == c)[0]
        p_c = pos[sel]
        e_t = np.zeros((S, EPC), np.float32)
        e_t[:, p_c] = e[sel].T
        xs_t = np.zeros((F_IN, EPC), NPBF16)
        xs_t[:, p_c] = x[edge_src[sel]].T.astype(NPBF16)
        dflat = np.full(EPC, -1.0, np.float32)
        dflat[p_c] = (edge_dst[sel] - c * NPC) % 128
        dlocv = np.ascontiguousarray(
            dflat.reshape(EPC // 128, 128).T).astype(np.float32)
        xown = np.zeros((KR, NPC_PAD), np.float32)
        xown[:F_IN, :NPC] = x[c * NPC:(c + 1) * NPC].T
        if use_ecc_bias:
            xown[F_IN, :NPC] = 1.0
        in_maps.append(dict(
            e_t=e_t, xs_t=xs_t, dloc=dlocv, iota_in=iota, xown_t=xown,
            w1rep=w1rep, w2r=w2r,
            b1rep=b1rep.astype(np.float32),
            b2r=b2r, rootk=rootk,
        ))
    return in_maps, EPC


def kernel(x, e, edge_src, edge_dst, W1, b1, W2, b2, root_kernel,
           ecc_bias, gat_kernel, attn_self, attn_neigh, gat_bias,
           _trace=False, _stats=None):
    x = np.asarray(x, np.float32)
    e = np.asarray(e, np.float32)
    edge_src = np.asarray(edge_src, np.int32)
    edge_dst = np.asarray(edge_dst, np.int32)
    W1 = np.asarray(W1, np.float32)
    b1 = np.asarray(b1, np.float32)
    W2 = np.asarray(W2, np.float32)
    b2 = np.asarray(b2, np.float32)
    root_kernel = np.asarray(root_kernel, np.float32)
    ecc_bias = np.asarray(ecc_bias, np.float32)
    gat_kernel = np.asarray(gat_kernel, np.float32)

    use_b1 = bool(np.any(b1))
    use_b2 = bool(np.any(b2))
    use_ecc_bias = bool(np.any(ecc_bias))

    in_maps, EPC = _prep_phase1_inputs(
        x, e, edge_src, edge_dst, W1, b1, W2, b2, root_kernel, ecc_bias,
        use_ecc_bias)
    gkrep = np.ascontiguousarray(
        np.broadcast_to(gat_kernel[:, 0, 0][None, :], (128, F_OUT))
    ).astype(np.float32)
    for m in in_maps:
        m["gkrep"] = gkrep

    nc1 = build_phase1(EPC, use_b1, use_b2, use_ecc_bias)
    res1 = run_bass_kernel_spmd(
        nc1, in_maps, core_ids=list(range(N_CORES)), trace=_trace)

    # xp_full[n] for all (padded) nodes
    xp_full = np.zeros(N, np.float32)
    for c in range(N_CORES):
        xp_c = res1.results[c]["xp_out"].T.reshape(NPC_PAD)[:NPC]
        xp_full[c * NPC:(c + 1) * NPC] = xp_c

    # ---- host halo-gather into fixed-degree slots ----
    indeg = np.bincount(edge_dst, minlength=N)
    D = _round_up(int(indeg.max()) + 1, 4)
    order2 = np.argsort(edge_dst, kind="stable")
    dsts = edge_dst[order2]
    srcs = edge_src[order2]
    seg_starts = np.searchsorted(dsts, np.arange(N))
    rank = np.arange(E_TOTAL) - seg_starts[dsts]
    slot = dsts.astype(np.int64) * D + 1 + rank

    xps_all = np.zeros(N * D, np.float32)
    mask_all = np.zeros(N * D, np.float32)
    xps_all[np.arange(N, dtype=np.int64) * D] = xp_full      # self loops
    mask_all[np.arange(N, dtype=np.int64) * D] = 1.0
    xps_all[slot] = xp_full[srcs]
    mask_all[slot] = 1.0

    cs = float(attn_self.reshape(-1)[0])
    cn = float(attn_neigh.reshape(-1)[0])
    gb = float(np.asarray(gat_bias).reshape(-1)[0])

    in_maps2 = []
    for c in range(N_CORES):
        xps_c = np.zeros((NPC_PAD, D), np.float32)
        mask_c = np.zeros((NPC_PAD, D), np.float32)
        xps_c[:NPC] = xps_all[c * NPC * D:(c + 1) * NPC * D].reshape(NPC, D)
        mask_c[:NPC] = mask_all[c * NPC * D:(c + 1) * NPC * D].reshape(NPC, D)
        # pad nodes: keep self slot live so denom != 0
        mask_c[NPC:, 0] = 1.0
        xps2 = xps_c.reshape(NT, 128, D).transpose(1, 0, 2).reshape(128, NT * D)
        mask2 = mask_c.reshape(NT, 128, D).transpose(1, 0, 2).reshape(128, NT * D)
        xpd = np.zeros((NPC_PAD,), np.float32)
        xpd[:NPC] = xp_full[c * NPC:(c + 1) * NPC]
        xpdcs = (cs * xpd).reshape(NT, 128).T
        in_maps2.append(dict(
            xps=np.ascontiguousarray(xps2),
            mask=np.ascontiguousarray(mask2),
            xpdcs=np.ascontiguousarray(xpdcs),
        ))

    nc2 = build_phase2(D, cs, cn, gb)
    res2 = run_bass_kernel_spmd(
        nc2, in_maps2, core_ids=list(range(N_CORES)), trace=_trace)

    out = np.zeros((N, 1), np.float32)
    for c in range(N_CORES):
        sig = res2.results[c]["sig_out"].T.reshape(NPC_PAD)[:NPC]
        out[c * NPC:(c + 1) * NPC, 0] = sig

    if _stats is not None:
        _stats["exec1_ns"] = res1.exec_time_ns
        _stats["exec2_ns"] = res2.exec_time_ns
        _stats["EPC"] = EPC
        _stats["D"] = D
        _stats["res1"] = res1
        _stats["res2"] = res2
    return out
